# revision 9
# baseline (speedup 1.0000x reference)
"""HypergraphConv (node->edge->node message passing) on 8 Trainium2 NeuronCores.

Self-contained Trainium kernel for:
    xw   = x @ W
    m_e  = (1/deg_e) * sum_{k: edge[k]=e} xw[src[k]]
    o_i  = (1/deg_i) * sum_{k: src[k]=i} m_{edge[k]} + bias
    out  = mean_i relu(o_i)                       # [128]

Sharding: nodes are split across the 8 cores (6250 each). Each core owns the
incidence entries whose src node falls in its shard; those entries drive both
the node->edge scatter (partial m, ReduceScatter'd across cores) and the
edge->node scatter (complete rows for the core's nodes, using the AllGather'd
bf16 m table).

Scatters are one-hot matmuls over sorted-and-padded entry streams; gathers use
the SWDGE dma_gather engine against bf16 row tables (256-byte rows). Degrees
(deg_e, deg_n) are precomputed on the host. The node->edge reduction is
ReduceScatter'd in slices pipelined against the scatter itself, scaled by
1/deg_e locally, and AllGather'd back as bf16.
"""

import os
import numpy as np
import ml_dtypes
from contextlib import ExitStack

import concourse.bacc as bacc
import concourse.bass as bass
import concourse.mybir as mybir
import concourse.tile as tile
from concourse import library_config
from concourse.bass_utils import run_bass_kernel_spmd

NCORES = 8
P = 128

N_NODES = 50000
N_EDGES = 20000
IN_DIM = 256
OUT_DIM = 128

BF16 = mybir.dt.bfloat16
F32 = mybir.dt.float32
I16 = mybir.dt.int16

PAD_OH = 200.0  # one-hot index for padding entries: matches no iota column
GCAP = 96       # max chunks per gather supergroup / dma_gather call


def _derived():
    npc = N_NODES // NCORES
    n_node_tiles = (npc + P - 1) // P
    raw = (N_EDGES + P - 1) // P
    nslice = 4 if raw >= 32 else 2
    align = NCORES * nslice
    et = ((raw + align - 1) // align) * align
    return npc, n_node_tiles, et, nslice


def _wrap_idx16(idx):
    """[L] int -> [128, L//16] int16 SWDGE index layout (16-wrap, x8 replicas)."""
    a = np.asarray(idx, dtype=np.int16).reshape(-1, 16).T
    return np.ascontiguousarray(np.tile(a, (8, 1)))


def _oh_cols(oh):
    """[L] float -> [128, L//128] bf16: column c holds entries c*128..c*128+127."""
    return np.ascontiguousarray(oh.reshape(-1, P).T.astype(ml_dtypes.bfloat16))


def _bucket_entries(gidx, tid, n_tiles, chunks, pad_row):
    """Lay out (gather idx, one-hot idx) entry streams grouped by tile.

    gidx: per-entry gather row index; tid: per-entry tile id;
    chunks[t]: number of 128-entry chunks allotted to tile t (static,
    shared across cores). Returns (gather_idx[L], onehot[L]) padded streams.
    """
    order = np.argsort(tid, kind="stable")
    gidx = gidx[order]
    tid_s = tid[order]
    counts = np.bincount(tid_s, minlength=n_tiles)
    starts = np.concatenate([[0], np.cumsum(counts[:-1])])
    dest_base = np.concatenate([[0], np.cumsum(chunks[:-1])]) * P
    L = int(chunks.sum()) * P
    g_out = np.full(L, pad_row, dtype=np.int64)
    oh_out = np.full(L, PAD_OH, dtype=np.float32)
    n = gidx.shape[0]
    rank = np.arange(n, dtype=np.int64) - starts[tid_s]
    dest = dest_base[tid_s] + rank
    g_out[dest] = gidx
    return g_out, oh_out, dest, order, L


def _groups(chunks, boundary=None):
    """Pack consecutive tiles into gather supergroups of <= GCAP chunks.
    Groups never span a `boundary`-tile multiple (for collective slicing).
    Returns list of (tile_lo, tile_hi, chunk_base, total_chunks)."""
    out = []
    t, cbase, n = 0, 0, len(chunks)
    while t < n:
        tot = int(chunks[t])
        hi = t + 1
        while (hi < n and tot + int(chunks[hi]) <= GCAP
               and not (boundary and hi % boundary == 0)):
            tot += int(chunks[hi])
            hi += 1
        out.append((t, hi, cbase, tot))
        cbase += tot
        t = hi
    return out


def build_kernel(chunks1, chunks2, last_nt):
    """Build the SPMD device program.

    chunks1[t]: #chunks for edge tile t (phase 1); chunks2[tt]: #chunks for
    node tile tt (phase 2); last_nt: node count of the last node tile.
    """
    npc, n_node_tiles, et, nslice = _derived()
    assert len(chunks1) == et and len(chunks2) == n_node_tiles
    tps = et // nslice              # edge tiles per collective slice
    tpcs = tps // NCORES            # tiles per core per slice (RS shard)
    rows_slice = tps * P
    rows_cs = tpcs * P
    tpc = tpcs * nslice             # this core's total scaled tiles
    np_rows = n_node_tiles * P
    LA = int(np.sum(chunks1)) * P
    LB = int(np.sum(chunks2)) * P
    NCA, NCB = LA // P, LB // P
    KMAX = int(max(chunks1.max(), chunks2.max()))
    GA = _groups(chunks1, boundary=tps)
    GB = _groups(chunks2)
    GDIMA = max(g[3] for g in GA)
    GDIMB = max(g[3] for g in GB)
    is_eq = mybir.AluOpType.is_equal
    COPY = mybir.ActivationFunctionType.Copy

    no_cc = os.environ.get("DBG_NO_CC") == "1"

    nc = bacc.Bacc("TRN2", num_devices=NCORES)

    xT_in = nc.dram_tensor("xT", [IN_DIM, npc], BF16, kind="ExternalInput")
    w_in = nc.dram_tensor("w", [IN_DIM, OUT_DIM], BF16, kind="ExternalInput")
    bias_in = nc.dram_tensor("bias", [1, OUT_DIM], F32, kind="ExternalInput")
    degb_in = nc.dram_tensor("degb", [1, np_rows], BF16, kind="ExternalInput")
    dinv_in = nc.dram_tensor("dinv", [P, n_node_tiles], F32, kind="ExternalInput")
    binv_in = nc.dram_tensor("binv", [P, tpc], F32, kind="ExternalInput")
    idxA_in = nc.dram_tensor("idxA", [P, LA // 16], I16, kind="ExternalInput")
    ohA_in = nc.dram_tensor("ohA", [P, NCA], BF16, kind="ExternalInput")
    idxB_in = nc.dram_tensor("idxB", [P, LB // 16], I16, kind="ExternalInput")
    ohB_in = nc.dram_tensor("ohB", [P, NCB], BF16, kind="ExternalInput")
    out_part = nc.dram_tensor("out_part", [OUT_DIM, 1], F32, kind="ExternalOutput")

    xwhl = nc.dram_tensor("xwhl", [np_rows, OUT_DIM], BF16)
    m_part = nc.dram_tensor("m_part", [et * P, OUT_DIM], BF16)
    shard = nc.dram_tensor("shard", [nslice * rows_cs, OUT_DIM], BF16)
    mshard = nc.dram_tensor("mshard", [nslice * rows_cs, OUT_DIM], BF16)
    mtab = nc.dram_tensor("mtab", [et * P, OUT_DIM], BF16, addr_space="Shared")

    with tile.TileContext(nc) as tc, ExitStack() as ctx:
        pin = ctx.enter_context(tc.tile_pool(name="pin", bufs=1))

        nc.gpsimd.load_library(library_config.mlp)

        # ---- persistent small tiles -------------------------------------
        iota_i = pin.tile([P, P * KMAX], I16)
        nc.gpsimd.iota(iota_i[:], [[1, P], [0, KMAX]], channel_multiplier=0)
        iotak = pin.tile([P, P, KMAX], BF16)  # iotak[p, j, c] = j
        nc.vector.tensor_copy(
            out=iotak[:].rearrange("p a b -> p (a b)"), in_=iota_i[:])
        ones_f32 = pin.tile([P, 1], F32)
        nc.vector.memset(ones_f32[:], 1.0)
        acc = pin.tile([P, OUT_DIM], F32)
        nc.vector.memset(acc[:], 0.0)
        bias_f = pin.tile([1, OUT_DIM], F32)
        nc.sync.dma_start(out=bias_f[:], in_=bias_in[:])
        bias_bf = pin.tile([1, OUT_DIM], BF16)
        nc.vector.tensor_copy(out=bias_bf[:], in_=bias_f[:])
        degb = pin.tile([1, np_rows], BF16)
        nc.sync.dma_start(out=degb[:], in_=degb_in[:])
        dinv = pin.tile([P, n_node_tiles], F32)
        nc.sync.dma_start(out=dinv[:], in_=dinv_in[:])
        binv = pin.tile([P, tpc], F32)
        nc.sync.dma_start(out=binv[:], in_=binv_in[:])
        # index/one-hot streams (loaded up front, used by phases 1/2)
        idxA = pin.tile([P, LA // 16], I16)
        ohA = pin.tile([P, NCA], BF16)
        idxB = pin.tile([P, LB // 16], I16)
        ohB = pin.tile([P, NCB], BF16)
        nc.sync.dma_start(out=idxA[:], in_=idxA_in[:])
        nc.sync.dma_start(out=ohA[:], in_=ohA_in[:])
        nc.sync.dma_start(out=idxB[:], in_=idxB_in[:])
        nc.sync.dma_start(out=ohB[:], in_=ohB_in[:])

        def s_build(S2, oh_tile, col0, kt):
            """S2[p, j, c] = (oh[p, col0+c] == j); packed inner dim -> fast DVE."""
            o = oh_tile[:, col0:col0 + kt]
            in0 = bass.AP(o.tensor, o.offset,
                          [list(o.ap[0]), [0, P], list(o.ap[1])])
            nc.vector.tensor_tensor(
                out=S2[:, :, 0:kt], in0=in0, in1=iotak[:, :, 0:kt], op=is_eq)

        # ---- stage A: xw = x @ W, bf16 table ----------------------------
        kh = IN_DIM // P
        with tc.tile_pool(name="pa", bufs=1) as pa, \
             tc.tile_pool(name="pa2", bufs=3) as pa2, \
             tc.tile_pool(name="psa", bufs=2, space="PSUM") as psa:
            xT_sb = [pa.tile([P, npc], BF16, tag=f"xT{k}", name=f"xT{k}")
                     for k in range(kh)]
            w_sb = [pa.tile([P, OUT_DIM], BF16, tag=f"w{k}", name=f"wsb{k}")
                    for k in range(kh)]
            for k in range(kh):
                nc.sync.dma_start(out=xT_sb[k][:], in_=xT_in[k * P:(k + 1) * P, :])
                nc.sync.dma_start(out=w_sb[k][:], in_=w_in[k * P:(k + 1) * P, :])
            XB = 4  # xwhl write batch (tiles)
            for i0 in range(0, n_node_tiles, XB):
                ib = min(XB, n_node_tiles - i0)
                st = pa2.tile([P, XB, OUT_DIM], BF16, tag="xst")
                for i in range(i0, i0 + ib):
                    r0 = i * P
                    nt = min(P, npc - r0)
                    pxw = psa.tile([P, OUT_DIM], F32, tag="pxw")
                    for k in range(kh):
                        nc.tensor.matmul(
                            out=pxw[:nt], lhsT=xT_sb[k][:, r0:r0 + nt],
                            rhs=w_sb[k][:], start=(k == 0), stop=(k == kh - 1))
                    nc.scalar.activation(
                        out=st[:, i - i0, :], in_=pxw[:], func=COPY)
                dst = bass.AP(xwhl, i0 * P * OUT_DIM,
                              [[OUT_DIM, P], [P * OUT_DIM, ib], [1, OUT_DIM]])
                nc.sync.dma_start(out=dst, in_=st[:, 0:ib, :])

        # ---- phase 1 scatter (node -> edge) + sliced RS/scale/AG --------
        with tc.tile_pool(name="pb", bufs=3) as pb, \
             tc.tile_pool(name="pbs", bufs=3) as pbs, \
             tc.tile_pool(name="psb", bufs=2, space="PSUM") as psb:

            GT1 = max(g[1] - g[0] for g in GA)

            def p1_group(lo, hi, cbase, tot):
                G = pb.tile([P, GDIMA, OUT_DIM], BF16, tag="G")
                for g0 in range(0, tot, GCAP):
                    gk = min(GCAP, tot - g0)
                    nc.gpsimd.dma_gather(
                        G[:, g0:g0 + gk, :], xwhl[:, :],
                        idxA[:, (cbase + g0) * 8:(cbase + g0 + gk) * 8],
                        gk * P, gk * P, OUT_DIM, single_packet=False)
                mst = pbs.tile([P, GT1, OUT_DIM], BF16, tag="mt")
                lb = 0
                for t in range(lo, hi):
                    kt = int(chunks1[t])
                    S2 = pb.tile([P, P, KMAX], BF16, tag="S")
                    s_build(S2, ohA, cbase + lb, kt)
                    pm = psb.tile([P, OUT_DIM], F32, tag="pm")
                    for c in range(kt):
                        nc.tensor.matmul(
                            out=pm[:], lhsT=S2[:, :, c], rhs=G[:, lb + c, :],
                            start=(c == 0), stop=(c == kt - 1),
                            skip_group_check=True)
                    nc.scalar.activation(out=mst[:, t - lo, :], in_=pm[:], func=COPY)
                    lb += kt
                dst = bass.AP(m_part, lo * P * OUT_DIM,
                              [[OUT_DIM, P], [P * OUT_DIM, hi - lo], [1, OUT_DIM]])
                nc.sync.dma_start(out=dst, in_=mst[:, 0:hi - lo, :])

            def rs_scale_ag(q):
                r0 = q * rows_slice
                s0 = q * rows_cs
                if no_cc:
                    nc.scalar.dma_start(out=shard[s0:s0 + rows_cs, :],
                                        in_=m_part[r0:r0 + rows_cs, :])
                else:
                    nc.gpsimd.collective_compute(
                        "ReduceScatter", mybir.AluOpType.add,
                        replica_groups=[list(range(NCORES))],
                        ins=[m_part[r0:r0 + rows_slice, :]],
                        outs=[shard[s0:s0 + rows_cs, :]])
                for st in range(tpcs):
                    ti = q * tpcs + st
                    sh = pbs.tile([P, OUT_DIM], BF16, tag="sh")
                    nc.scalar.dma_start(out=sh[:], in_=shard[ti * P:(ti + 1) * P, :])
                    shb = pbs.tile([P, OUT_DIM], BF16, tag="shb")
                    nc.scalar.activation(out=shb[:], in_=sh[:], func=COPY,
                                         scale=binv[:, ti:ti + 1])
                    nc.scalar.dma_start(out=mshard[ti * P:(ti + 1) * P, :],
                                        in_=shb[:])
                if no_cc:
                    nc.scalar.dma_start(out=mtab[r0:r0 + rows_cs, :],
                                        in_=mshard[s0:s0 + rows_cs, :])
                else:
                    nc.gpsimd.collective_compute(
                        "AllGather", mybir.AluOpType.bypass,
                        replica_groups=[list(range(NCORES))],
                        ins=[mshard[s0:s0 + rows_cs, :]],
                        outs=[mtab[r0:r0 + rows_slice, :]])

            q = 0
            for (lo, hi, cbase, tot) in GA:
                p1_group(lo, hi, cbase, tot)
                while q < nslice and hi >= (q + 1) * tps:
                    rs_scale_ag(q)
                    q += 1
            while q < nslice:
                rs_scale_ag(q)
                q += 1

        # ---- phase 2 scatter (edge -> node) + post ----------------------
        with tc.tile_pool(name="pd", bufs=3) as pd, \
             tc.tile_pool(name="pd2", bufs=3) as pd2, \
             tc.tile_pool(name="psd", bufs=2, space="PSUM") as psd:
            for (lo, hi, cbase, tot) in GB:
                G2 = pd.tile([P, GDIMB, OUT_DIM], BF16, tag="G2")
                for g0 in range(0, tot, GCAP):
                    gk = min(GCAP, tot - g0)
                    nc.gpsimd.dma_gather(
                        G2[:, g0:g0 + gk, :], mtab[:, :],
                        idxB[:, (cbase + g0) * 8:(cbase + g0 + gk) * 8],
                        gk * P, gk * P, OUT_DIM, single_packet=False)
                lb = 0
                for tt in range(lo, hi):
                    kt = int(chunks2[tt])
                    nt = last_nt if tt == n_node_tiles - 1 else P
                    S2 = pd.tile([P, P, KMAX], BF16, tag="S2")
                    s_build(S2, ohB, cbase + lb, kt)
                    po = psd.tile([P, OUT_DIM], F32, tag="po")
                    # bias * deg folded into the PSUM group: after the final
                    # 1/deg scale this adds exactly `bias` per node row.
                    nc.tensor.matmul(
                        out=po[:], lhsT=degb[0:1, tt * P:(tt + 1) * P],
                        rhs=bias_bf[:], start=True, stop=False,
                        skip_group_check=True)
                    for c in range(kt):
                        nc.tensor.matmul(
                            out=po[:], lhsT=S2[:, :, c], rhs=G2[:, lb + c, :],
                            start=False, stop=(c == kt - 1),
                            skip_group_check=True)
                    ot = pd2.tile([P, OUT_DIM], F32, tag="ot")
                    nc.vector.tensor_scalar(
                        out=ot[:nt], in0=po[:nt], scalar1=dinv[:nt, tt:tt + 1],
                        scalar2=0.0, op0=mybir.AluOpType.mult,
                        op1=mybir.AluOpType.max)
                    nc.vector.tensor_tensor(
                        out=acc[:nt], in0=acc[:nt], in1=ot[:nt],
                        op=mybir.AluOpType.add)
                    lb += kt

        # ---- final: column sum over nodes -> [OUT_DIM, 1] ---------------
        with tc.tile_pool(name="pe", bufs=1) as pe, \
             tc.tile_pool(name="pse", bufs=1, space="PSUM") as pse:
            pcol = pse.tile([P, 1], F32)
            nc.tensor.matmul(out=pcol[:OUT_DIM], lhsT=acc[:], rhs=ones_f32[:],
                             start=True, stop=True)
            ocol = pe.tile([P, 1], F32)
            nc.vector.tensor_copy(out=ocol[:OUT_DIM], in_=pcol[:OUT_DIM])
            nc.sync.dma_start(out=out_part[:, :], in_=ocol[:OUT_DIM])

    nc.compile()
    return nc


def prepare_inputs(x, w, bias, hyperedge_index):
    """Host-side sharding: split entries by src-node shard, sort/pad both
    phase streams, compute the static chunk structure and degree tables."""
    npc, n_node_tiles, et, nslice = _derived()
    tps = et // nslice
    tpcs = tps // NCORES
    tpc = tpcs * nslice
    np_rows = n_node_tiles * P
    src = np.asarray(hyperedge_index[0], dtype=np.int64)
    edge = np.asarray(hyperedge_index[1], dtype=np.int64)

    # global degree tables
    deg_e = np.bincount(edge, minlength=et * P).astype(np.float64)
    b_inv = (1.0 / np.maximum(deg_e, 1.0)).astype(np.float32)
    b_inv_t = b_inv.reshape(et, P).T  # [P, et]
    deg_n = np.bincount(src, minlength=N_NODES).astype(np.float64)

    core_of = src // npc
    per_core = []
    for c in range(NCORES):
        sel = core_of == c
        per_core.append((src[sel] - c * npc, edge[sel]))

    # static chunk structure = max over cores, per tile
    cnt1 = np.zeros((NCORES, et), np.int64)
    cnt2 = np.zeros((NCORES, n_node_tiles), np.int64)
    for c, (s_loc, e_glob) in enumerate(per_core):
        cnt1[c] = np.bincount(e_glob // P, minlength=et)
        cnt2[c] = np.bincount(s_loc // P, minlength=n_node_tiles)
    chunks1 = np.maximum(1, -(-cnt1.max(axis=0) // P))
    chunks2 = np.maximum(1, -(-cnt2.max(axis=0) // P))

    in_maps = []
    for c, (s_loc, e_glob) in enumerate(per_core):
        # phase 1: group by edge tile; gather xwhl[s_loc], one-hot = edge%P
        t1 = e_glob // P
        g1, oh1, dest1, order1, LA = _bucket_entries(s_loc, t1, et, chunks1, 0)
        oh1[dest1] = (e_glob % P)[order1].astype(np.float32)
        # phase 2: group by node tile; gather mtab[e_glob], one-hot = s_loc%P
        t2 = s_loc // P
        g2, oh2, dest2, order2, LB = _bucket_entries(
            e_glob, t2, n_node_tiles, chunks2, 0)
        oh2[dest2] = (s_loc % P)[order2].astype(np.float32)

        # degree tables for this core's node shard
        dn = np.zeros(np_rows, np.float64)
        dn[:npc] = deg_n[c * npc:(c + 1) * npc]
        dinv = (1.0 / np.maximum(dn, 1.0)).astype(np.float32)
        degb = np.maximum(dn, 1.0).astype(np.float32)
        # b_inv columns for this core's RS shards, in (slice, tile) order
        bcols = np.concatenate(
            [b_inv_t[:, q * tps + c * tpcs:q * tps + (c + 1) * tpcs]
             for q in range(nslice)], axis=1)

        xT = np.ascontiguousarray(
            x[c * npc:(c + 1) * npc].T.astype(ml_dtypes.bfloat16))
        in_maps.append({
            "xT": xT,
            "w": np.ascontiguousarray(w.astype(ml_dtypes.bfloat16)),
            "bias": np.ascontiguousarray(bias.astype(np.float32)).reshape(1, -1),
            "degb": np.ascontiguousarray(
                degb.astype(ml_dtypes.bfloat16)).reshape(1, -1),
            "dinv": np.ascontiguousarray(dinv.reshape(n_node_tiles, P).T),
            "binv": np.ascontiguousarray(bcols),
            "idxA": _wrap_idx16(g1),
            "ohA": _oh_cols(oh1),
            "idxB": _wrap_idx16(g2),
            "ohB": _oh_cols(oh2),
        })

    last_nt = npc - (n_node_tiles - 1) * P
    return in_maps, chunks1, chunks2, last_nt


def kernel(x_node_features, lin_weight, bias, hyperedge_index):
    in_maps, chunks1, chunks2, last_nt = prepare_inputs(
        x_node_features, lin_weight, bias, hyperedge_index)
    nc = build_kernel(chunks1, chunks2, last_nt)
    res = run_bass_kernel_spmd(nc, in_maps, list(range(NCORES)))
    total = np.zeros(OUT_DIM, np.float64)
    for c in range(NCORES):
        total += res.results[c]["out_part"][:, 0].astype(np.float64)
    return (total / N_NODES).astype(np.float32)


# revision 17
# speedup vs baseline: 2.1032x; 2.1032x over previous
"""HypergraphConv (node->edge->node message passing) on 8 Trainium2 NeuronCores.

Self-contained Trainium kernel for:
    xw   = x @ W
    m_e  = (1/deg_e) * sum_{k: edge[k]=e} xw[src[k]]
    o_i  = (1/deg_i) * sum_{k: src[k]=i} m_{edge[k]} + bias
    out  = mean_i relu(o_i)                       # [128]

Sharding: nodes are split across the 8 cores (6250 each). Each core owns the
incidence entries whose src node falls in its shard; those entries drive both
the node->edge scatter (partial m, ReduceScatter'd across cores) and the
edge->node scatter (complete rows for the core's nodes, using the AllGather'd
bf16 m table).

Scatters are one-hot matmuls over sorted-and-padded entry streams; gathers use
the SWDGE dma_gather engine against bf16 row tables (256-byte rows). Degrees
(deg_e, deg_n) are precomputed on the host. The node->edge reduction is
ReduceScatter'd in slices pipelined against the scatter itself, scaled by
1/deg_e locally, and AllGather'd back as bf16.
"""

import os
import numpy as np
import ml_dtypes
from contextlib import ExitStack

import concourse.bacc as bacc
import concourse.bass as bass
import concourse.mybir as mybir
import concourse.tile as tile
from concourse import library_config
from concourse.bass_utils import run_bass_kernel_spmd

NCORES = 8
P = 128

N_NODES = 50000
N_EDGES = 20000
IN_DIM = 256
OUT_DIM = 128

BF16 = mybir.dt.bfloat16
F32 = mybir.dt.float32
I16 = mybir.dt.int16

PAD_OH = 200.0  # one-hot index for padding entries: matches no iota column
GCAP = 64       # max chunks per gather supergroup / dma_gather call
NQ = 4          # SWDGE queues (ucode max 4); gathers rotate across them


def _derived():
    npc = N_NODES // NCORES
    n_node_tiles = (npc + P - 1) // P
    raw = (N_EDGES + P - 1) // P
    nslice = int(os.environ.get("DBG_NSLICE", "0")) or (4 if raw >= 32 else 2)
    align = NCORES * nslice
    et = ((raw + align - 1) // align) * align
    return npc, n_node_tiles, et, nslice


def _wrap_idx16(idx):
    """[L] int -> [128, L//16] int16 SWDGE index layout (16-wrap, x8 replicas)."""
    a = np.asarray(idx, dtype=np.int16).reshape(-1, 16).T
    return np.ascontiguousarray(np.tile(a, (8, 1)))


def _oh_cols(oh):
    """[L] float -> [128, L//128] bf16: column c holds entries c*128..c*128+127."""
    return np.ascontiguousarray(oh.reshape(-1, P).T.astype(ml_dtypes.bfloat16))


def _dedup_slots(gidx, ohv, tid):
    """Merge pairs of same-(tile, gidx) entries into slots carrying up to two
    one-hot targets. Returns per-slot (tid, gidx, oh0, oh1, isdup)."""
    n = gidx.shape[0]
    key = tid.astype(np.int64) * (1 << 32) + gidx.astype(np.int64)
    order = np.argsort(key, kind="stable")
    k_s = key[order]
    g_s = gidx[order]
    t_s = tid[order]
    o_s = ohv[order]
    new_grp = np.empty(n, bool)
    new_grp[0] = True
    new_grp[1:] = k_s[1:] != k_s[:-1]
    first_pos = np.maximum.accumulate(np.where(new_grp, np.arange(n), 0))
    pos_in_grp = np.arange(n) - first_pos
    pidx = np.flatnonzero(pos_in_grp % 2 == 0)
    s_t, s_g, s_oh0 = t_s[pidx], g_s[pidx], o_s[pidx]
    has2 = np.zeros(len(pidx), bool)
    nxt = pidx + 1
    ok = nxt < n
    has2[ok] = k_s[nxt[ok]] == k_s[pidx[ok]]
    s_oh1 = np.full(len(pidx), PAD_OH, np.float32)
    s_oh1[has2] = o_s[nxt[has2]]
    return s_t, s_g, s_oh0, s_oh1, has2


def _slot_counts(s_t, isdup, n_tiles):
    return (np.bincount(s_t, minlength=n_tiles),
            np.bincount(s_t[isdup], minlength=n_tiles))


def _bucket_slots(s_t, s_g, s_oh0, s_oh1, isdup, n_tiles, chunks, kdup):
    """Place slots into the padded per-tile streams, dup slots first.

    Returns (g[L], oh0[L], oh1[KD*128]) where dup one-hots live in the
    compact per-tile kdup-chunk stream."""
    order = np.argsort(s_t * 2 + (~isdup).astype(np.int64), kind="stable")
    t_s = s_t[order]
    counts = np.bincount(t_s, minlength=n_tiles)
    starts = np.concatenate([[0], np.cumsum(counts[:-1])])
    rank = np.arange(len(t_s), dtype=np.int64) - starts[t_s]
    dest_base = np.concatenate([[0], np.cumsum(chunks[:-1])]) * P
    L = int(chunks.sum()) * P
    g_out = np.zeros(L, dtype=np.int64)
    oh0_out = np.full(L, PAD_OH, dtype=np.float32)
    dest = dest_base[t_s] + rank
    g_out[dest] = s_g[order]
    oh0_out[dest] = s_oh0[order]
    KD = int(kdup.sum())
    oh1_out = np.full(max(KD, 1) * P, PAD_OH, dtype=np.float32)
    dup_base = np.concatenate([[0], np.cumsum(kdup[:-1])]) * P
    sel = isdup[order]
    oh1_out[dup_base[t_s[sel]] + rank[sel]] = s_oh1[order][sel]
    return g_out, oh0_out, oh1_out, L


def _groups(chunks, boundary=None):
    """Pack consecutive tiles into gather supergroups of <= GCAP chunks.
    Groups never span a `boundary`-tile multiple (for collective slicing).
    Returns list of (tile_lo, tile_hi, chunk_base, total_chunks)."""
    out = []
    t, cbase, n = 0, 0, len(chunks)
    while t < n:
        tot = int(chunks[t])
        hi = t + 1
        while (hi < n and tot + int(chunks[hi]) <= GCAP
               and not (boundary and hi % boundary == 0)):
            tot += int(chunks[hi])
            hi += 1
        out.append((t, hi, cbase, tot))
        cbase += tot
        t = hi
    return out


def build_kernel(chunks1, chunks2, last_nt):
    """Build the SPMD device program.

    chunks1[t]: #chunks for edge tile t (phase 1); chunks2[tt]: #chunks for
    node tile tt (phase 2); last_nt: node count of the last node tile.
    """
    npc, n_node_tiles, et, nslice = _derived()
    chunks1, kdup1 = np.asarray(chunks1)
    chunks2, kdup2 = np.asarray(chunks2)
    assert len(chunks1) == et and len(chunks2) == n_node_tiles
    tps = et // nslice              # edge tiles per collective slice
    tpcs = tps // NCORES            # tiles per core per slice (RS shard)
    rows_slice = tps * P
    rows_cs = tpcs * P
    tpc = tpcs * nslice             # this core's total scaled tiles
    np_rows = n_node_tiles * P
    LA = int(np.sum(chunks1)) * P
    LB = int(np.sum(chunks2)) * P
    NCA, NCB = LA // P, LB // P
    KMAX = int(max(chunks1.max(), chunks2.max()))
    KD1 = max(int(kdup1.sum()), 1)
    KD2 = max(int(kdup2.sum()), 1)
    KDMAX = max(int(kdup1.max()), int(kdup2.max()), 1)
    dbase1 = np.concatenate([[0], np.cumsum(kdup1[:-1])])
    dbase2 = np.concatenate([[0], np.cumsum(kdup2[:-1])])
    GA = _groups(chunks1, boundary=tps)
    GB = _groups(chunks2)
    GDIMA = max(g[3] for g in GA)
    GDIMB = max(g[3] for g in GB)
    is_eq = mybir.AluOpType.is_equal
    COPY = mybir.ActivationFunctionType.Copy

    no_cc = os.environ.get("DBG_NO_CC") == "1"

    nc = bacc.Bacc("TRN2", num_devices=NCORES, num_swdge_queues=NQ)

    xT_in = nc.dram_tensor("xT", [IN_DIM, npc], BF16, kind="ExternalInput")
    w_in = nc.dram_tensor("w", [IN_DIM, OUT_DIM], BF16, kind="ExternalInput")
    bias_in = nc.dram_tensor("bias", [1, OUT_DIM], F32, kind="ExternalInput")
    degb_in = nc.dram_tensor("degb", [1, np_rows], BF16, kind="ExternalInput")
    dinv_in = nc.dram_tensor("dinv", [P, n_node_tiles], F32, kind="ExternalInput")
    binv_in = nc.dram_tensor("binv", [P, tpc], F32, kind="ExternalInput")
    idxA_in = nc.dram_tensor("idxA", [P, LA // 16], I16, kind="ExternalInput")
    ohA_in = nc.dram_tensor("ohA", [P, NCA], BF16, kind="ExternalInput")
    oh1A_in = nc.dram_tensor("oh1A", [P, KD1], BF16, kind="ExternalInput")
    idxB_in = nc.dram_tensor("idxB", [P, LB // 16], I16, kind="ExternalInput")
    ohB_in = nc.dram_tensor("ohB", [P, NCB], BF16, kind="ExternalInput")
    oh1B_in = nc.dram_tensor("oh1B", [P, KD2], BF16, kind="ExternalInput")
    out_part = nc.dram_tensor("out_part", [OUT_DIM, 1], F32, kind="ExternalOutput")

    xwhl = nc.dram_tensor("xwhl", [np_rows, OUT_DIM], BF16)
    m_part = nc.dram_tensor("m_part", [et * P, OUT_DIM], BF16)
    shard = nc.dram_tensor("shard", [nslice * rows_cs, OUT_DIM], BF16)
    mshard = nc.dram_tensor("mshard", [nslice * rows_cs, OUT_DIM], BF16)
    mtab = nc.dram_tensor("mtab", [et * P, OUT_DIM], BF16, addr_space="Shared")

    with tile.TileContext(nc) as tc, ExitStack() as ctx:
        pin = ctx.enter_context(tc.tile_pool(name="pin", bufs=1))

        nc.gpsimd.load_library(library_config.mlp)

        # ---- persistent small tiles -------------------------------------
        iota_i = pin.tile([P, P * KMAX], I16)
        nc.gpsimd.iota(iota_i[:], [[1, P], [0, KMAX]], channel_multiplier=0)
        iotak = pin.tile([P, P, KMAX], BF16)  # iotak[p, j, c] = j
        nc.vector.tensor_copy(
            out=iotak[:].rearrange("p a b -> p (a b)"), in_=iota_i[:])
        ones_f32 = pin.tile([P, 1], F32)
        nc.vector.memset(ones_f32[:], 1.0)
        acc = pin.tile([P, OUT_DIM], F32)
        nc.vector.memset(acc[:], 0.0)
        bias_f = pin.tile([1, OUT_DIM], F32)
        nc.sync.dma_start(out=bias_f[:], in_=bias_in[:])
        bias_bf = pin.tile([1, OUT_DIM], BF16)
        nc.vector.tensor_copy(out=bias_bf[:], in_=bias_f[:])
        degb = pin.tile([1, np_rows], BF16)
        nc.sync.dma_start(out=degb[:], in_=degb_in[:])
        dinv = pin.tile([P, n_node_tiles], F32)
        nc.sync.dma_start(out=dinv[:], in_=dinv_in[:])
        binv = pin.tile([P, tpc], F32)
        nc.sync.dma_start(out=binv[:], in_=binv_in[:])
        # index/one-hot streams (loaded up front, used by phases 1/2)
        idxA = pin.tile([P, LA // 16], I16)
        ohA = pin.tile([P, NCA], BF16)
        oh1A = pin.tile([P, KD1], BF16)
        idxB = pin.tile([P, LB // 16], I16)
        ohB = pin.tile([P, NCB], BF16)
        oh1B = pin.tile([P, KD2], BF16)
        nc.sync.dma_start(out=idxA[:], in_=idxA_in[:])
        nc.sync.dma_start(out=ohA[:], in_=ohA_in[:])
        nc.sync.dma_start(out=oh1A[:], in_=oh1A_in[:])
        nc.sync.dma_start(out=idxB[:], in_=idxB_in[:])
        nc.sync.dma_start(out=ohB[:], in_=ohB_in[:])
        nc.sync.dma_start(out=oh1B[:], in_=oh1B_in[:])

        qctr = [0]

        def next_q():
            q = qctr[0] % NQ
            qctr[0] += 1
            return q

        def make_S(pool, tag):
            return pool.tile([P, P, KMAX], BF16, tag=tag, name=tag)

        def s_build(S2, oh_tile, col0, kt):
            """S2[p, j, c] = (oh[p, col0+c] == j); packed inner dim -> fast DVE."""
            o = oh_tile[:, col0:col0 + kt]
            in0 = bass.AP(o.tensor, o.offset,
                          [list(o.ap[0]), [0, P], list(o.ap[1])])
            nc.vector.tensor_tensor(
                out=S2[:, :, 0:kt], in0=in0, in1=iotak[:, :, 0:kt], op=is_eq)

        def s_build_dup(S2, Tt, oh1_tile, db, kd):
            """Second one-hot pass for merged duplicate slots (chunks [0,kd))."""
            o = oh1_tile[:, db:db + kd]
            in0 = bass.AP(o.tensor, o.offset,
                          [list(o.ap[0]), [0, P], list(o.ap[1])])
            nc.vector.tensor_tensor(
                out=Tt[:, :, 0:kd], in0=in0, in1=iotak[:, :, 0:kd], op=is_eq)
            nc.vector.tensor_tensor(
                out=S2[:, :, 0:kd], in0=S2[:, :, 0:kd], in1=Tt[:, :, 0:kd],
                op=mybir.AluOpType.add)

        def s_lhsT(S2, c):
            return S2[:, :, c]

        # ---- stage A: xw = x @ W, bf16 table ----------------------------
        kh = IN_DIM // P
        with tc.tile_pool(name="pa", bufs=1) as pa, \
             tc.tile_pool(name="pa2", bufs=3) as pa2, \
             tc.tile_pool(name="psa", bufs=2, space="PSUM") as psa:
            xT_sb = [pa.tile([P, npc], BF16, tag=f"xT{k}", name=f"xT{k}")
                     for k in range(kh)]
            w_sb = [pa.tile([P, OUT_DIM], BF16, tag=f"w{k}", name=f"wsb{k}")
                    for k in range(kh)]
            for k in range(kh):
                nc.sync.dma_start(out=xT_sb[k][:], in_=xT_in[k * P:(k + 1) * P, :])
                nc.sync.dma_start(out=w_sb[k][:], in_=w_in[k * P:(k + 1) * P, :])
            XB = 4  # xwhl write batch (tiles)
            for i0 in range(0, n_node_tiles, XB):
                ib = min(XB, n_node_tiles - i0)
                st = pa2.tile([P, XB, OUT_DIM], BF16, tag="xst")
                for i in range(i0, i0 + ib):
                    r0 = i * P
                    nt = min(P, npc - r0)
                    pxw = psa.tile([P, OUT_DIM], F32, tag="pxw")
                    for k in range(kh):
                        nc.tensor.matmul(
                            out=pxw[:nt], lhsT=xT_sb[k][:, r0:r0 + nt],
                            rhs=w_sb[k][:], start=(k == 0), stop=(k == kh - 1))
                    nc.scalar.activation(
                        out=st[:, i - i0, :], in_=pxw[:], func=COPY)
                dst = bass.AP(xwhl, i0 * P * OUT_DIM,
                              [[OUT_DIM, P], [P * OUT_DIM, ib], [1, OUT_DIM]])
                nc.sync.dma_start(out=dst, in_=st[:, 0:ib, :])

        # ---- phase 1 scatter (node -> edge) + sliced RS/scale/AG --------
        with tc.tile_pool(name="pb", bufs=4) as pb, \
             tc.tile_pool(name="pbS", bufs=3) as pbS, \
             tc.tile_pool(name="pbs", bufs=3) as pbs, \
             tc.tile_pool(name="psb", bufs=2, space="PSUM") as psb:

            GT1 = max(g[1] - g[0] for g in GA)

            def p1_group(lo, hi, cbase, tot):
                G = pb.tile([P, GDIMA, OUT_DIM], BF16, tag="G")
                for g0 in range(0, tot, GCAP):
                    gk = min(GCAP, tot - g0)
                    nc.gpsimd.dma_gather(
                        G[:, g0:g0 + gk, :], xwhl[:, :],
                        idxA[:, (cbase + g0) * 8:(cbase + g0 + gk) * 8],
                        gk * P, gk * P, OUT_DIM, single_packet=False,
                        queue_num=next_q())
                mst = pbs.tile([P, GT1, OUT_DIM], BF16, tag="mt")
                lb = 0
                for t in range(lo, hi):
                    kt = int(chunks1[t])
                    S2 = make_S(pbS, "S")
                    s_build(S2, ohA, cbase + lb, kt)
                    kd = int(kdup1[t])
                    if kd:
                        Tt = pbS.tile([P, P, KDMAX], BF16, tag="T", name="T")
                        s_build_dup(S2, Tt, oh1A, int(dbase1[t]), kd)
                    pm = psb.tile([P, OUT_DIM], F32, tag="pm")
                    for c in range(kt):
                        nc.tensor.matmul(
                            out=pm[:], lhsT=s_lhsT(S2, c), rhs=G[:, lb + c, :],
                            start=(c == 0), stop=(c == kt - 1),
                            skip_group_check=True)
                    nc.scalar.activation(out=mst[:, t - lo, :], in_=pm[:], func=COPY)
                    lb += kt
                dst = bass.AP(m_part, lo * P * OUT_DIM,
                              [[OUT_DIM, P], [P * OUT_DIM, hi - lo], [1, OUT_DIM]])
                nc.sync.dma_start(out=dst, in_=mst[:, 0:hi - lo, :])

            def rs_scale_ag(q):
                r0 = q * rows_slice
                s0 = q * rows_cs
                if no_cc:
                    nc.scalar.dma_start(out=shard[s0:s0 + rows_cs, :],
                                        in_=m_part[r0:r0 + rows_cs, :])
                else:
                    nc.gpsimd.collective_compute(
                        "ReduceScatter", mybir.AluOpType.add,
                        replica_groups=[list(range(NCORES))],
                        ins=[m_part[r0:r0 + rows_slice, :]],
                        outs=[shard[s0:s0 + rows_cs, :]])
                for st in range(tpcs):
                    ti = q * tpcs + st
                    sh = pbs.tile([P, OUT_DIM], BF16, tag="sh")
                    nc.scalar.dma_start(out=sh[:], in_=shard[ti * P:(ti + 1) * P, :])
                    shb = pbs.tile([P, OUT_DIM], BF16, tag="shb")
                    nc.scalar.activation(out=shb[:], in_=sh[:], func=COPY,
                                         scale=binv[:, ti:ti + 1])
                    nc.scalar.dma_start(out=mshard[ti * P:(ti + 1) * P, :],
                                        in_=shb[:])
                if no_cc:
                    nc.scalar.dma_start(out=mtab[r0:r0 + rows_cs, :],
                                        in_=mshard[s0:s0 + rows_cs, :])
                else:
                    nc.gpsimd.collective_compute(
                        "AllGather", mybir.AluOpType.bypass,
                        replica_groups=[list(range(NCORES))],
                        ins=[mshard[s0:s0 + rows_cs, :]],
                        outs=[mtab[r0:r0 + rows_slice, :]])

            q = 0
            for (lo, hi, cbase, tot) in GA:
                p1_group(lo, hi, cbase, tot)
                while q < nslice and hi >= (q + 1) * tps:
                    rs_scale_ag(q)
                    q += 1
            while q < nslice:
                rs_scale_ag(q)
                q += 1

        # ---- phase 2 scatter (edge -> node) + post ----------------------
        with tc.tile_pool(name="pd", bufs=4) as pd, \
             tc.tile_pool(name="pdS", bufs=3) as pdS, \
             tc.tile_pool(name="pd2", bufs=3) as pd2, \
             tc.tile_pool(name="psd", bufs=2, space="PSUM") as psd:
            for (lo, hi, cbase, tot) in GB:
                G2 = pd.tile([P, GDIMB, OUT_DIM], BF16, tag="G2")
                for g0 in range(0, tot, GCAP):
                    gk = min(GCAP, tot - g0)
                    nc.gpsimd.dma_gather(
                        G2[:, g0:g0 + gk, :], mtab[:, :],
                        idxB[:, (cbase + g0) * 8:(cbase + g0 + gk) * 8],
                        gk * P, gk * P, OUT_DIM, single_packet=False,
                        queue_num=next_q())
                lb = 0
                for tt in range(lo, hi):
                    kt = int(chunks2[tt])
                    nt = last_nt if tt == n_node_tiles - 1 else P
                    S2 = make_S(pdS, "S2")
                    s_build(S2, ohB, cbase + lb, kt)
                    kd = int(kdup2[tt])
                    if kd:
                        Tt = pdS.tile([P, P, KDMAX], BF16, tag="T2", name="T2")
                        s_build_dup(S2, Tt, oh1B, int(dbase2[tt]), kd)
                    po = psd.tile([P, OUT_DIM], F32, tag="po")
                    # bias * deg folded into the PSUM group: after the final
                    # 1/deg scale this adds exactly `bias` per node row.
                    nc.tensor.matmul(
                        out=po[:], lhsT=degb[0:1, tt * P:(tt + 1) * P],
                        rhs=bias_bf[:], start=True, stop=False,
                        skip_group_check=True)
                    for c in range(kt):
                        nc.tensor.matmul(
                            out=po[:], lhsT=s_lhsT(S2, c), rhs=G2[:, lb + c, :],
                            start=False, stop=(c == kt - 1),
                            skip_group_check=True)
                    ot = pd2.tile([P, OUT_DIM], F32, tag="ot")
                    nc.vector.tensor_scalar(
                        out=ot[:nt], in0=po[:nt], scalar1=dinv[:nt, tt:tt + 1],
                        scalar2=0.0, op0=mybir.AluOpType.mult,
                        op1=mybir.AluOpType.max)
                    nc.vector.tensor_tensor(
                        out=acc[:nt], in0=acc[:nt], in1=ot[:nt],
                        op=mybir.AluOpType.add)
                    lb += kt

        # ---- final: column sum over nodes -> [OUT_DIM, 1] ---------------
        with tc.tile_pool(name="pe", bufs=1) as pe, \
             tc.tile_pool(name="pse", bufs=1, space="PSUM") as pse:
            pcol = pse.tile([P, 1], F32)
            nc.tensor.matmul(out=pcol[:OUT_DIM], lhsT=acc[:], rhs=ones_f32[:],
                             start=True, stop=True)
            ocol = pe.tile([P, 1], F32)
            nc.vector.tensor_copy(out=ocol[:OUT_DIM], in_=pcol[:OUT_DIM])
            nc.sync.dma_start(out=out_part[:, :], in_=ocol[:OUT_DIM])

    nc.compile()
    return nc


def prepare_inputs(x, w, bias, hyperedge_index):
    """Host-side sharding: split entries by src-node shard, sort/pad both
    phase streams, compute the static chunk structure and degree tables."""
    npc, n_node_tiles, et, nslice = _derived()
    tps = et // nslice
    tpcs = tps // NCORES
    tpc = tpcs * nslice
    np_rows = n_node_tiles * P
    src = np.asarray(hyperedge_index[0], dtype=np.int64)
    edge = np.asarray(hyperedge_index[1], dtype=np.int64)

    # global degree tables
    deg_e = np.bincount(edge, minlength=et * P).astype(np.float64)
    b_inv = (1.0 / np.maximum(deg_e, 1.0)).astype(np.float32)
    b_inv_t = b_inv.reshape(et, P).T  # [P, et]
    deg_n = np.bincount(src, minlength=N_NODES).astype(np.float64)

    core_of = src // npc
    per_core = []
    for c in range(NCORES):
        sel = core_of == c
        per_core.append((src[sel] - c * npc, edge[sel]))

    # dedup'd slot streams per core; static chunk structure = max over cores
    slots1, slots2 = [], []
    cnt1 = np.zeros((NCORES, et), np.int64)
    dct1 = np.zeros((NCORES, et), np.int64)
    cnt2 = np.zeros((NCORES, n_node_tiles), np.int64)
    dct2 = np.zeros((NCORES, n_node_tiles), np.int64)
    for c, (s_loc, e_glob) in enumerate(per_core):
        # phase 1: tile by edge; gather xwhl[s_loc]; one-hot = edge%P
        sl1 = _dedup_slots(s_loc, (e_glob % P).astype(np.float32), e_glob // P)
        cnt1[c], dct1[c] = _slot_counts(sl1[0], sl1[4], et)
        slots1.append(sl1)
        # phase 2: tile by node; gather mtab[e_glob]; one-hot = s_loc%P
        sl2 = _dedup_slots(e_glob, (s_loc % P).astype(np.float32), s_loc // P)
        cnt2[c], dct2[c] = _slot_counts(sl2[0], sl2[4], n_node_tiles)
        slots2.append(sl2)
    chunks1 = np.maximum(1, -(-cnt1.max(axis=0) // P))
    chunks2 = np.maximum(1, -(-cnt2.max(axis=0) // P))
    kdup1 = -(-dct1.max(axis=0) // P)
    kdup2 = -(-dct2.max(axis=0) // P)

    in_maps = []
    for c in range(NCORES):
        g1, oh1a, oh1b, LA = _bucket_slots(*slots1[c], et, chunks1, kdup1)
        g2, oh2a, oh2b, LB = _bucket_slots(
            *slots2[c], n_node_tiles, chunks2, kdup2)
        s_loc, e_glob = per_core[c]

        # degree tables for this core's node shard
        dn = np.zeros(np_rows, np.float64)
        dn[:npc] = deg_n[c * npc:(c + 1) * npc]
        dinv = (1.0 / np.maximum(dn, 1.0)).astype(np.float32)
        degb = np.maximum(dn, 1.0).astype(np.float32)
        # b_inv columns for this core's RS shards, in (slice, tile) order
        bcols = np.concatenate(
            [b_inv_t[:, q * tps + c * tpcs:q * tps + (c + 1) * tpcs]
             for q in range(nslice)], axis=1)

        xT = np.ascontiguousarray(
            x[c * npc:(c + 1) * npc].T.astype(ml_dtypes.bfloat16))
        in_maps.append({
            "xT": xT,
            "w": np.ascontiguousarray(w.astype(ml_dtypes.bfloat16)),
            "bias": np.ascontiguousarray(bias.astype(np.float32)).reshape(1, -1),
            "degb": np.ascontiguousarray(
                degb.astype(ml_dtypes.bfloat16)).reshape(1, -1),
            "dinv": np.ascontiguousarray(dinv.reshape(n_node_tiles, P).T),
            "binv": np.ascontiguousarray(bcols),
            "idxA": _wrap_idx16(g1),
            "ohA": _oh_cols(oh1a),
            "oh1A": _oh_cols(oh1b),
            "idxB": _wrap_idx16(g2),
            "ohB": _oh_cols(oh2a),
            "oh1B": _oh_cols(oh2b),
        })

    last_nt = npc - (n_node_tiles - 1) * P
    return (in_maps, np.stack([chunks1, kdup1]), np.stack([chunks2, kdup2]),
            last_nt)


def kernel(x_node_features, lin_weight, bias, hyperedge_index):
    in_maps, chunks1, chunks2, last_nt = prepare_inputs(
        x_node_features, lin_weight, bias, hyperedge_index)
    nc = build_kernel(chunks1, chunks2, last_nt)
    res = run_bass_kernel_spmd(nc, in_maps, list(range(NCORES)))
    total = np.zeros(OUT_DIM, np.float64)
    for c in range(NCORES):
        total += res.results[c]["out_part"][:, 0].astype(np.float64)
    return (total / N_NODES).astype(np.float32)
